# revision 2
# baseline (speedup 1.0000x reference)
"""Trainium2 Bass kernel for EquivariantLieConvLayer (GNN message passing).

Math (exact restructuring):
  reference computes, per edge e = (s -> t):
      msg_e = alpha_bil * bracket(alpha_msg * F[s], F[t]);  agg[t] += msg_e
      out = F + agg + update_scale * bracket(agg, alpha_w * agg)
  bracket is bilinear and F[t] is shared by all edges targeting t, so
      agg[t] = alpha_bil*alpha_msg * bracket(S[t], F[t]),
      S[t] = sum_{e->t} F[src_e]   (plain scatter-add of source rows)
  and bracket(x, a*x) == 0 exactly (antisymmetrized structure constants),
  so out = F + agg (the +F is applied host-side in f32).

Device mapping (8 cores, SPMD, no collectives): target nodes are binned
host-side into 8 cores x 40 windows of 64 slots, balancing per-window
in-edge counts so every core runs an identical instruction stream.

Key design points (measured on HW):
  * gathered source rows ship as fp8 e3m4 (1 B/elem, bit-exact vs
    ml_dtypes on the PE): halves the dominant DMA stream.  End-to-end rel
    err ~1.35e-2 vs the 2e-2 gate (verified in numpy beforehand).
  * per-group one-hot H matrices (edge -> slot-in-window) also ship as
    e3m4, packed with G into one [G(256)|H(64)] record per group, one DMA
    per 8-window chunk: scatter is plain matmul streams, no on-chip H gen.
  * scatter: PSUM quads [128, 8*64] (h0|h1 halves of 4 windows in one
    PSUM bank; each accumulation group kept contiguous), ACT copies to a
    persistent bf16 S^T.
  * bracket: per 512-node chunk, Gx = Q^T S^T (Q one-hot, fp8), terms =
    Gx * fgy on DVE (fgy = host-gathered F[:, cj] in bf16, chunk-major
    DMA), agg^T = P^T terms (P carries cv * scale in bf16).  The 600
    triples are permuted by (ck-block, gray(ci-block)) so ~half the
    (t-block, h) weight blocks are all-zero and their streams skipped.
  * software pipeline two quads deep: gx(cn) issues after scatter quad
    2cn+3, p(cn) one period later, so the ACT sT copies and DVE terms are
    never on the PE critical path; the PE stays busy enough to hold the
    2.4 GHz p-state.  The last chunk is split into two 256-column halves
    with finely interleaved gx/p streams to shorten the drain tail.
"""

import numpy as np
import ml_dtypes

import concourse.bass as bass
import concourse.tile as tile
from concourse import bacc, mybir
from concourse.bass_utils import run_bass_kernel_spmd

BF16 = mybir.dt.bfloat16
F32 = mybir.dt.float32
E3 = mybir.dt.float8e3

N_NODES = 20000
D = 248
D_PAD = 256
N_CORES = 8
W = 64                            # target slots per scatter window
N_CPAD = 2560                     # padded node slots per core
N_WIN = N_CPAD // W               # 40
NNZ = 600
TS = 640                          # padded t dim (5 blocks of 128)
NT = TS // 128                    # 5
CW = 512                          # bracket chunk width (node columns)
NCH = N_CPAD // CW                # 5 chunks
WPC = CW // W                     # 8 windows per chunk

e3 = ml_dtypes.float8_e3m4
e4 = ml_dtypes.float8_e4m3
bf = ml_dtypes.bfloat16

_CACHE = {}


def _build(g_w, qmask, pmask):
    """g_w[w] = #128-edge groups for window w (same on all cores).
    qmask[h][m] / pmask[m][h]: whether the Q / P weight block is nonzero."""
    tot_g = int(sum(g_w))
    g_off = np.concatenate([[0], np.cumsum(g_w)]).astype(int)
    # DMA granularity: one G/H load per quad (4 windows)
    n_quads = N_WIN // 4
    ch_g = [(int(g_off[q * 4]), int(g_off[(q + 1) * 4]))
            for q in range(n_quads)]

    nc = bacc.Bacc("TRN2", target_bir_lowering=False, debug=False,
                   num_devices=N_CORES)

    GHW = D_PAD + W                   # packed per-group width (G | H)
    ghr = nc.dram_tensor("ghr", [128, tot_g * GHW], E3, kind="ExternalInput")
    qarrd = nc.dram_tensor("qarr", [128, 2 * TS], E3, kind="ExternalInput")
    cpbd = nc.dram_tensor("cpb", [128, NT * D_PAD], BF16,
                          kind="ExternalInput")
    fgy = nc.dram_tensor("fgy", [128, NCH * NT * CW], BF16, kind="ExternalInput")
    out_d = nc.dram_tensor("out", [128, 2 * N_CPAD], BF16, kind="ExternalOutput")

    o_p = 0

    with tile.TileContext(nc) as tc:
        with tc.tile_pool(name="sb", bufs=1) as cpool, \
             tc.tile_pool(name="psum", bufs=1, space="PSUM") as pp:
            gpool = cpool
            wpool = cpool

            max_cg = max(g1 - g0 for g0, g1 in ch_g)
            g_tiles = []
            h_tiles = []
            grp_tile = {}

            def load_quad_gh(q):
                g0, g1 = ch_g[q]
                cg = g1 - g0
                gh_t = gpool.tile([128, max_cg * GHW], E3, tag="GH", bufs=6,
                                  name=f"GH{q}")
                nc.sync.dma_start(out=gh_t[:, :cg * GHW],
                                  in_=ghr.ap()[:, g0 * GHW:g1 * GHW])
                g_tiles.append(gh_t)
                for g in range(g0, g1):
                    grp_tile[g] = (gh_t, gh_t, g - g0)

            fgy_sb = [cpool.tile([128, NT * CW], BF16, tag=f"fgyc{cn}",
                                 name=f"fgyc{cn}") for cn in range(NCH)]

            def load_fgy(cn):
                nc.sync.dma_start(
                    out=fgy_sb[cn][:],
                    in_=fgy.ap()[:, cn * NT * CW:(cn + 1) * NT * CW])

            # DMA order: quad-granular G/H first (scatter starts early),
            # weights after the first two quads, fgy per chunk just ahead
            # of its gx phase.
            load_quad_gh(0)
            load_quad_gh(1)
            qarr = cpool.tile([128, 2 * TS], E3, tag="qarr")
            nc.sync.dma_start(out=qarr[:], in_=qarrd.ap())
            cpb = cpool.tile([128, NT * D_PAD], BF16, tag="cpb")
            nc.sync.dma_start(out=cpb[:], in_=cpbd.ap())
            load_fgy(0)
            for q2 in range(2, n_quads):
                load_quad_gh(q2)
                if q2 % 2 == 1 and q2 // 2 < NCH:
                    load_fgy(q2 // 2)

            # persistent S^T, feature-major, 2 h-planes
            sT = [cpool.tile([128, N_CPAD], BF16, tag=f"sT{h}", name=f"sT{h}")
                  for h in range(2)]

            def scatter_quad(q):
                """Scatter 4 windows into one packed PSUM tile (h0|h1 halves,
                one PSUM bank) and copy both halves to sT."""
                ps = pp.tile([128, 8 * W], F32, tag="swin", bufs=2,
                             name=f"ps{q}", padded_shape=[128, 8 * W])
                for wi in range(4):
                    w = 4 * q + wi
                    gw = int(g_w[w])
                    # keep each PSUM accumulation group contiguous: finish
                    # h0's group before starting h1's (same PSUM bank)
                    for h in range(2):
                        for g in range(gw):
                            g_t, h_t, slot = grp_tile[g_off[w] + g]
                            nc.tensor.matmul(
                                out=ps[:, h * 4 * W + wi * W:
                                       h * 4 * W + (wi + 1) * W],
                                lhsT=g_t[:, slot * GHW + h * 128:
                                         slot * GHW + (h + 1) * 128],
                                rhs=h_t[:, slot * GHW + D_PAD:
                                        slot * GHW + D_PAD + W],
                                start=(g == 0), stop=(g == gw - 1),
                            )
                # both copies on ACT: DVE is the busiest engine (terms), and
                # anything queued behind terms there stalls the next gx phase
                for h in range(2):
                    nc.scalar.copy(out=sT[h][:, q * 4 * W:(q + 1) * 4 * W],
                                   in_=ps[:, h * 4 * W:(h + 1) * 4 * W])

            chunk_state = {}
            chunk_po = {}

            # P streams: for h, over m with pmask[m][h]
            p_sched = {h: [m for m in range(NT) if pmask[m][h]]
                       for h in range(2)}

            def p_matmuls(po, terms, m):
                for h in range(2):
                    if not pmask[m][h]:
                        continue
                    seq = p_sched[h]
                    nc.tensor.matmul(
                        out=po[h][:],
                        lhsT=cpb[:, o_p + m * D_PAD + h * 128:
                                o_p + m * D_PAD + (h + 1) * 128],
                        rhs=terms[m][:],
                        start=(m == seq[0]), stop=(m == seq[-1]))

            def alloc_po(key, cw=CW):
                return [pp.tile([128, cw], F32, tag=f"out{h}", bufs=1,
                                name=f"po{key}_{h}", padded_shape=[128, CW])
                        for h in range(2)]

            def gx_phase(cn, fine=False, off=0, cw=CW, key=None):
                key = cn if key is None else key
                c0 = cn * CW + off
                nsl = slice(c0, c0 + cw)
                terms = [None] * NT
                chunk_state[key] = terms

                def gx_term(m):
                    hs = [h for h in range(2) if qmask[h][m]]
                    pt = pp.tile([128, cw], F32, tag="gxy", bufs=4,
                                 name=f"gxp{key}_{m}", padded_shape=[128, CW])
                    for i, h in enumerate(hs):
                        nc.tensor.matmul(
                            out=pt[:],
                            lhsT=qarr[:, h * TS + m * 128:
                                      h * TS + (m + 1) * 128],
                            rhs=sT[h][:, nsl],
                            start=(i == 0), stop=(i == len(hs) - 1))
                    tm = wpool.tile([128, cw], BF16, tag=f"terms{m}",
                                    bufs=2, name=f"terms{m}_{key}",
                                    padded_shape=[128, CW])
                    nc.vector.tensor_tensor(
                        out=tm[:], in0=pt[:],
                        in1=fgy_sb[cn][:, m * CW + off:m * CW + off + cw],
                        op=mybir.AluOpType.mult)
                    terms[m] = tm

                if not fine:
                    for m in range(NT):
                        gx_term(m)
                else:
                    # tail chunk: interleave P streams to overlap the DVE
                    # terms latency (no scatter work left to hide behind)
                    po = alloc_po(key, cw)
                    chunk_po[key] = po
                    gx_term(0)
                    gx_term(1)
                    gx_term(2)
                    p_matmuls(po, terms, 0)
                    gx_term(3)
                    p_matmuls(po, terms, 1)
                    gx_term(4)
                    for m in range(2, NT):
                        p_matmuls(po, terms, m)

            def p_phase(cn, off=0, cw=CW, key=None):
                key = cn if key is None else key
                c0 = cn * CW + off
                terms = chunk_state.pop(key)
                if key in chunk_po:
                    po = chunk_po.pop(key)
                else:
                    po = alloc_po(key, cw)
                    for m in range(NT):
                        p_matmuls(po, terms, m)
                osb = wpool.tile([128, 2 * cw], BF16, tag="osb", bufs=2,
                                 name=f"osb{key}", padded_shape=[128, 2 * CW])
                oap = out_d.ap()
                for h in range(2):
                    nc.scalar.copy(out=osb[:, h * cw:h * cw + cw],
                                   in_=po[h][:])
                nc.sync.dma_start(
                    out=bass.AP(oap.tensor, oap.offset + c0,
                                [[2 * N_CPAD, 128], [N_CPAD, 2], [1, cw]]),
                    in_=osb[:, :2 * cw])

            # software pipeline, two quads deep: gx(cn) issues after scatter
            # quad 2cn+3, so the ACT sT copies of quad 2cn+1 (which gate it)
            # run underneath scatter quads 2cn+2 / 2cn+3 -- the PE never
            # waits on them.  p(cn) one period later.  The last chunk has no
            # scatter to hide behind, so interleave its gx/p streams finely.
            for q in range(n_quads):
                scatter_quad(q)
                if q % 2 == 1 and q >= 3:
                    gx_phase((q - 3) // 2)
                    if q >= 5:
                        p_phase((q - 5) // 2)
            p_phase(NCH - 2)
            hw_ = CW // 2
            gx_phase(NCH - 1, fine=True, off=0, cw=hw_, key="4a")
            p_phase(NCH - 1, off=0, cw=hw_, key="4a")
            gx_phase(NCH - 1, fine=True, off=hw_, cw=hw_, key="4b")
            p_phase(NCH - 1, off=hw_, cw=hw_, key="4b")

    nc.compile()
    return nc


def _prep(features, edge_index, ci, cj, ck, cv,
          alpha_msg, alpha_bil, alpha_w, update_scale):
    F = np.asarray(features, np.float32)
    ei = np.asarray(edge_index)
    ci = np.asarray(ci); cj = np.asarray(cj); ck = np.asarray(ck)
    cv = np.asarray(cv, np.float32)
    am = float(alpha_msg); ab = float(alpha_bil)
    src, tgt = ei[0].astype(np.int64), ei[1].astype(np.int64)
    n_bins = N_CORES * N_WIN

    # --- balanced assignment of nodes to (core, window) bins ---
    deg = np.bincount(tgt, minlength=N_NODES)
    order = np.argsort(-deg, kind="stable")
    bin_load = np.zeros(n_bins, np.int64)
    bin_fill = np.zeros(n_bins, np.int64)
    node_bin = np.empty(N_NODES, np.int64)
    node_slot = np.empty(N_NODES, np.int64)
    import heapq
    heap = [(0, b) for b in range(n_bins)]
    heapq.heapify(heap)
    for n in order:
        while True:
            load, b = heapq.heappop(heap)
            if bin_fill[b] < W:
                break
        node_bin[n] = b
        node_slot[n] = bin_fill[b]
        bin_fill[b] += 1
        bin_load[b] = load + deg[n]
        if bin_fill[b] < W:
            heapq.heappush(heap, (int(bin_load[b]), b))
    g_w_all = np.ceil(bin_load.reshape(N_CORES, N_WIN) / 128).astype(np.int64)
    g_w = np.maximum(1, g_w_all.max(axis=0))
    tot_g = int(g_w.sum())
    g_offs = np.concatenate([[0], np.cumsum(g_w)]).astype(int)

    node_core = node_bin // N_WIN
    node_win = node_bin % N_WIN
    node_local = node_win * W + node_slot          # in [0, N_CPAD)

    # --- per-core edge slots ---
    e_core = node_core[tgt]
    tot_idx = tot_g * 128
    idx_all = np.zeros((N_CORES, tot_idx), np.int32)
    col_all = np.full((N_CORES, tot_idx), -1.0, np.float32)
    eorder = np.lexsort((tgt, node_win[tgt], e_core))
    src_s = src[eorder]; core_s = e_core[eorder]; win_s = node_win[tgt][eorder]
    tl_s = node_slot[tgt][eorder]                  # slot within window
    counts = np.zeros((N_CORES, N_WIN), np.int64)
    np.add.at(counts, (core_s, win_s), 1)
    run_starts = np.zeros((N_CORES, N_WIN), np.int64)
    np.cumsum(counts.ravel()[:-1], out=run_starts.ravel()[1:])
    for c in range(N_CORES):
        for w in range(N_WIN):
            cnt = int(counts[c, w]); s0 = int(run_starts[c, w])
            base = g_offs[w] * 128
            idx_all[c, base:base + cnt] = src_s[s0:s0 + cnt].astype(np.int32)
            col_all[c, base:base + cnt] = tl_s[s0:s0 + cnt].astype(np.float32)

    # --- t-axis permutation: sort triples by (ck-block, gray(ci-block)) so
    # adjacent classes share the ci-block at one boundary: fewest mixed
    # (t-block, h) weight blocks for both Q and P ---
    kb = (ck // 128).astype(np.int64)
    ib = (ci // 128).astype(np.int64)
    perm = np.lexsort((ib ^ kb, kb))
    cip, cjp, ckp = ci[perm], cj[perm], ck[perm]
    cvp = cv[perm]
    scale = ab * am
    Q1 = np.zeros((D_PAD, TS), np.float32)
    Q1[cip, np.arange(NNZ)] = 1.0
    P1 = np.zeros((TS, D_PAD), np.float32)
    P1[np.arange(NNZ), ckp] = cvp * scale
    qmask = [[bool(np.any(Q1[h * 128:(h + 1) * 128,
                             m * 128:(m + 1) * 128]))
              for m in range(NT)] for h in range(2)]
    pmask = [[bool(np.any(P1[m * 128:(m + 1) * 128,
                             h * 128:(h + 1) * 128]))
              for h in range(2)] for m in range(NT)]
    qp = [Q1[h * 128:(h + 1) * 128, :] for h in range(2)]
    qarr_h = np.concatenate(qp, axis=1).astype(e3)        # [128, 2*TS]
    pp_ = [P1[m * 128:(m + 1) * 128, :] for m in range(NT)]
    cpackb = np.concatenate(pp_, axis=1).astype(bf)       # [128, NT*D_PAD]

    # --- packed per-group [G(256) | H(64)] e3m4 tables ---
    GHW = D_PAD + W
    iota = np.arange(W, dtype=np.float32)

    # --- gathered source rows (e3m4) + per-core tensors ---
    fpad = np.zeros((N_NODES, D_PAD), e3)
    fpad[:, :D] = F.astype(e3)
    inv = np.full((N_CORES, N_CPAD), -1, np.int64)
    inv[node_core, node_local] = np.arange(N_NODES)
    Fbf = F.astype(bf)

    in_maps = []
    for c in range(N_CORES):
        gh = np.empty((tot_g, 128, GHW), e3)
        gh[:, :, :D_PAD] = fpad[idx_all[c]].reshape(tot_g, 128, D_PAD)
        tcols = col_all[c].reshape(tot_g, 128)     # [tot_g, 128]
        gh[:, :, D_PAD:] = (tcols[:, :, None]
                            == iota[None, None, :]).astype(e3)
        ghr_c = np.ascontiguousarray(
            gh.transpose(1, 0, 2).reshape(128, tot_g * GHW))
        sel = inv[c]
        valid = sel >= 0
        fv = F[sel[valid]]                         # [nvalid, D]
        fgy_c = np.zeros((TS, N_CPAD), bf)
        fgy_c[:NNZ, valid] = fv[:, cjp].T.astype(bf)
        # chunk-major: [128, NCH, NT, CW]
        fgy_r = fgy_c.reshape(NT, 128, NCH, CW)
        fgy_cm = np.ascontiguousarray(
            fgy_r.transpose(1, 2, 0, 3).reshape(128, NCH * NT * CW))
        in_maps.append({
            "ghr": ghr_c,
            "qarr": qarr_h,
            "cpb": cpackb,
            "fgy": fgy_cm,
        })
    return (tuple(g_w.tolist()),
            tuple(tuple(r) for r in qmask), tuple(tuple(r) for r in pmask),
            in_maps, inv, F)


def _run(in_maps, inv, fref, nc, trace=False):
    res = run_bass_kernel_spmd(nc, in_maps, core_ids=list(range(N_CORES)),
                               trace=trace)
    out = np.empty((N_NODES, D), np.float32)
    for c in range(N_CORES):
        sel = inv[c]
        valid = sel >= 0
        arr = res.results[c]["out"]              # [128, 2*N_CPAD] bf16 agg^T
        full = np.concatenate([arr[:, :N_CPAD], arr[:, N_CPAD:]], axis=0)
        out[sel[valid]] = (full[:D, valid].T.astype(np.float32)
                           + fref[sel[valid]])
    return out, res


def _get(inputs):
    g_w, qmask, pmask, in_maps, inv, fref = _prep(**inputs)
    key = (g_w, qmask, pmask)
    if key not in _CACHE:
        _CACHE[key] = _build(np.array(g_w), [list(r) for r in qmask],
                             [list(r) for r in pmask])
    return in_maps, inv, fref, _CACHE[key]


def kernel(**inputs):
    in_maps, inv, fref, nc = _get(inputs)
    out, _ = _run(in_maps, inv, fref, nc, trace=False)
    return out


def kernel_traced(**inputs):
    in_maps, inv, fref, nc = _get(inputs)
    return _run(in_maps, inv, fref, nc, trace=True)


# revision 4
# speedup vs baseline: 1.0754x; 1.0754x over previous
"""Trainium2 Bass kernel for EquivariantLieConvLayer (GNN message passing).

Math (exact restructuring):
  reference computes, per edge e = (s -> t):
      msg_e = alpha_bil * bracket(alpha_msg * F[s], F[t]);  agg[t] += msg_e
      out = F + agg + update_scale * bracket(agg, alpha_w * agg)
  bracket is bilinear and F[t] is shared by all edges targeting t, so
      agg[t] = alpha_bil*alpha_msg * bracket(S[t], F[t]),
      S[t] = sum_{e->t} F[src_e]   (plain scatter-add of source rows)
  and bracket(x, a*x) == 0 exactly (antisymmetrized structure constants),
  so out = F + agg (the +F is applied host-side in f32).

Device mapping (8 cores, SPMD, no collectives): target nodes are binned
host-side into 8 cores x 40 windows of 64 slots, balancing per-window
in-edge counts so every core runs an identical instruction stream.

Key design points (measured on HW):
  * gathered source rows ship as fp8 e3m4 (1 B/elem, bit-exact vs
    ml_dtypes on the PE): halves the dominant DMA stream.  End-to-end rel
    err ~1.35e-2 vs the 2e-2 gate (verified in numpy beforehand).
  * per-group one-hot H matrices (edge -> slot-in-window) also ship as
    e3m4, packed with G into one [G(256)|H(64)] record per group, one DMA
    per 8-window chunk: scatter is plain matmul streams, no on-chip H gen.
  * scatter: PSUM quads [128, 8*64] (h0|h1 halves of 4 windows in one
    PSUM bank; each accumulation group kept contiguous), ACT copies to a
    persistent bf16 S^T.
  * bracket: per 512-node chunk, Gx = Q^T S^T (Q one-hot, fp8), terms =
    Gx * fgy on DVE (fgy = host-gathered F[:, cj] in bf16, chunk-major
    DMA), agg^T = P^T terms (P carries cv * scale in bf16).  The 600
    triples are permuted by (ck-block, gray(ci-block)) so ~half the
    (t-block, h) weight blocks are all-zero and their streams skipped.
  * software pipeline one quad deep (A/B-measured best with the
    chunk-granular GH loads): gx(cn) issues right after its second quad,
    p(cn) after the following quad, keeping the PE stream dense enough to
    hold the 2.4 GHz p-state.
"""

import numpy as np
import ml_dtypes

import concourse.bass as bass
import concourse.tile as tile
from concourse import bacc, mybir
from concourse.bass_utils import run_bass_kernel_spmd

BF16 = mybir.dt.bfloat16
F32 = mybir.dt.float32
E3 = mybir.dt.float8e3

N_NODES = 20000
D = 248
D_PAD = 256
N_CORES = 8
W = 64                            # target slots per scatter window
N_CPAD = 2560                     # padded node slots per core
N_WIN = N_CPAD // W               # 40
NNZ = 600
TS = 640                          # padded t dim (5 blocks of 128)
NT = TS // 128                    # 5
CW = 512                          # bracket chunk width (node columns)
NCH = N_CPAD // CW                # 5 chunks
WPC = CW // W                     # 8 windows per chunk

e3 = ml_dtypes.float8_e3m4
e4 = ml_dtypes.float8_e4m3
bf = ml_dtypes.bfloat16

_CACHE = {}


def _build(g_w, qmask, pmask):
    """g_w[w] = #128-edge groups for window w (same on all cores).
    qmask[h][m] / pmask[m][h]: whether the Q / P weight block is nonzero."""
    tot_g = int(sum(g_w))
    g_off = np.concatenate([[0], np.cumsum(g_w)]).astype(int)
    # DMA granularity: one packed G/H load per 8-window chunk -- paired
    # with the 1-quad-deep schedule below this measured fastest (A/B'd
    # against quad-granular loads and deeper pipelines)
    n_quads = N_WIN // 4
    ch_g = [(int(g_off[cn * WPC]), int(g_off[(cn + 1) * WPC]))
            for cn in range(NCH)]

    nc = bacc.Bacc("TRN2", target_bir_lowering=False, debug=False,
                   num_devices=N_CORES)

    GHW = D_PAD + W                   # packed per-group width (G | H)
    ghr = nc.dram_tensor("ghr", [128, tot_g * GHW], E3, kind="ExternalInput")
    qarrd = nc.dram_tensor("qarr", [128, 2 * TS], E3, kind="ExternalInput")
    cpbd = nc.dram_tensor("cpb", [128, NT * D_PAD], BF16,
                          kind="ExternalInput")
    fgy = nc.dram_tensor("fgy", [128, NCH * NT * CW], BF16, kind="ExternalInput")
    out_d = nc.dram_tensor("out", [128, 2 * N_CPAD], BF16, kind="ExternalOutput")

    o_p = 0

    with tile.TileContext(nc) as tc:
        with tc.tile_pool(name="sb", bufs=1) as cpool, \
             tc.tile_pool(name="psum", bufs=1, space="PSUM") as pp:
            gpool = cpool
            wpool = cpool

            max_cg = max(g1 - g0 for g0, g1 in ch_g)
            g_tiles = []
            h_tiles = []
            grp_tile = {}

            def load_chunk_gh(c):
                g0, g1 = ch_g[c]
                cg = g1 - g0
                gh_t = gpool.tile([128, max_cg * GHW], E3, tag="GH", bufs=3,
                                  name=f"GH{c}")
                nc.sync.dma_start(out=gh_t[:, :cg * GHW],
                                  in_=ghr.ap()[:, g0 * GHW:g1 * GHW])
                g_tiles.append(gh_t)
                for g in range(g0, g1):
                    grp_tile[g] = (gh_t, gh_t, g - g0)

            fgy_sb = [cpool.tile([128, NT * CW], BF16, tag=f"fgyc{cn}",
                                 name=f"fgyc{cn}") for cn in range(NCH)]

            def load_fgy(cn):
                nc.sync.dma_start(
                    out=fgy_sb[cn][:],
                    in_=fgy.ap()[:, cn * NT * CW:(cn + 1) * NT * CW])

            # DMA order: GH chunk 0 first (scatter starts ASAP), weights,
            # then each later GH chunk followed by the previous chunk's fgy
            load_chunk_gh(0)
            qarr = cpool.tile([128, 2 * TS], E3, tag="qarr")
            nc.sync.dma_start(out=qarr[:], in_=qarrd.ap())
            cpb = cpool.tile([128, NT * D_PAD], BF16, tag="cpb")
            nc.sync.dma_start(out=cpb[:], in_=cpbd.ap())
            load_chunk_gh(1)
            load_fgy(0)
            load_chunk_gh(2)
            load_fgy(1)
            load_chunk_gh(3)
            load_fgy(2)
            load_chunk_gh(4)
            load_fgy(3)
            load_fgy(4)

            # persistent S^T, feature-major, 2 h-planes
            sT = [cpool.tile([128, N_CPAD], BF16, tag=f"sT{h}", name=f"sT{h}")
                  for h in range(2)]

            def scatter_quad(q):
                """Scatter 4 windows into one packed PSUM tile (h0|h1 halves,
                one PSUM bank) and copy both halves to sT."""
                ps = pp.tile([128, 8 * W], F32, tag="swin", bufs=2,
                             name=f"ps{q}", padded_shape=[128, 8 * W])
                for wi in range(4):
                    w = 4 * q + wi
                    gw = int(g_w[w])
                    # keep each PSUM accumulation group contiguous: finish
                    # h0's group before starting h1's (same PSUM bank)
                    for h in range(2):
                        for g in range(gw):
                            g_t, h_t, slot = grp_tile[g_off[w] + g]
                            nc.tensor.matmul(
                                out=ps[:, h * 4 * W + wi * W:
                                       h * 4 * W + (wi + 1) * W],
                                lhsT=g_t[:, slot * GHW + h * 128:
                                         slot * GHW + (h + 1) * 128],
                                rhs=h_t[:, slot * GHW + D_PAD:
                                        slot * GHW + D_PAD + W],
                                start=(g == 0), stop=(g == gw - 1),
                            )
                # both copies on ACT: DVE is the busiest engine (terms), and
                # anything queued behind terms there stalls the next gx phase
                for h in range(2):
                    nc.scalar.copy(out=sT[h][:, q * 4 * W:(q + 1) * 4 * W],
                                   in_=ps[:, h * 4 * W:(h + 1) * 4 * W])

            chunk_state = {}
            chunk_po = {}

            # P streams: for h, over m with pmask[m][h]
            p_sched = {h: [m for m in range(NT) if pmask[m][h]]
                       for h in range(2)}

            def p_matmuls(po, terms, m):
                for h in range(2):
                    if not pmask[m][h]:
                        continue
                    seq = p_sched[h]
                    nc.tensor.matmul(
                        out=po[h][:],
                        lhsT=cpb[:, o_p + m * D_PAD + h * 128:
                                o_p + m * D_PAD + (h + 1) * 128],
                        rhs=terms[m][:],
                        start=(m == seq[0]), stop=(m == seq[-1]))

            def alloc_po(key, cw=CW):
                return [pp.tile([128, cw], F32, tag=f"out{h}", bufs=1,
                                name=f"po{key}_{h}", padded_shape=[128, CW])
                        for h in range(2)]

            def gx_phase(cn, fine=False, off=0, cw=CW, key=None):
                key = cn if key is None else key
                c0 = cn * CW + off
                nsl = slice(c0, c0 + cw)
                terms = [None] * NT
                chunk_state[key] = terms

                def gx_term(m):
                    hs = [h for h in range(2) if qmask[h][m]]
                    pt = pp.tile([128, cw], F32, tag="gxy", bufs=4,
                                 name=f"gxp{key}_{m}", padded_shape=[128, CW])
                    for i, h in enumerate(hs):
                        nc.tensor.matmul(
                            out=pt[:],
                            lhsT=qarr[:, h * TS + m * 128:
                                      h * TS + (m + 1) * 128],
                            rhs=sT[h][:, nsl],
                            start=(i == 0), stop=(i == len(hs) - 1))
                    tm = wpool.tile([128, cw], BF16, tag=f"terms{m}",
                                    bufs=2, name=f"terms{m}_{key}",
                                    padded_shape=[128, CW])
                    nc.vector.tensor_tensor(
                        out=tm[:], in0=pt[:],
                        in1=fgy_sb[cn][:, m * CW + off:m * CW + off + cw],
                        op=mybir.AluOpType.mult)
                    terms[m] = tm

                if not fine:
                    for m in range(NT):
                        gx_term(m)
                else:
                    # tail chunk: interleave P streams to overlap the DVE
                    # terms latency (no scatter work left to hide behind)
                    po = alloc_po(key, cw)
                    chunk_po[key] = po
                    gx_term(0)
                    gx_term(1)
                    gx_term(2)
                    p_matmuls(po, terms, 0)
                    gx_term(3)
                    p_matmuls(po, terms, 1)
                    gx_term(4)
                    for m in range(2, NT):
                        p_matmuls(po, terms, m)

            def p_phase(cn, off=0, cw=CW, key=None):
                key = cn if key is None else key
                c0 = cn * CW + off
                terms = chunk_state.pop(key)
                if key in chunk_po:
                    po = chunk_po.pop(key)
                else:
                    po = alloc_po(key, cw)
                    for m in range(NT):
                        p_matmuls(po, terms, m)
                osb = wpool.tile([128, 2 * cw], BF16, tag="osb", bufs=2,
                                 name=f"osb{key}", padded_shape=[128, 2 * CW])
                oap = out_d.ap()
                for h in range(2):
                    nc.scalar.copy(out=osb[:, h * cw:h * cw + cw],
                                   in_=po[h][:])
                nc.sync.dma_start(
                    out=bass.AP(oap.tensor, oap.offset + c0,
                                [[2 * N_CPAD, 128], [N_CPAD, 2], [1, cw]]),
                    in_=osb[:, :2 * cw])

            # software pipeline, one quad deep (the measured-best depth
            # with chunk-granular GH loads): gx(cn) right after its second
            # quad, p(cn) after the following quad.
            for q in range(n_quads):
                scatter_quad(q)
                if q % 2 == 1:
                    gx_phase((q - 1) // 2)
                elif q >= 2:
                    p_phase(q // 2 - 1)
            p_phase(NCH - 1)

    nc.compile()
    return nc


def _prep(features, edge_index, ci, cj, ck, cv,
          alpha_msg, alpha_bil, alpha_w, update_scale):
    F = np.asarray(features, np.float32)
    ei = np.asarray(edge_index)
    ci = np.asarray(ci); cj = np.asarray(cj); ck = np.asarray(ck)
    cv = np.asarray(cv, np.float32)
    am = float(alpha_msg); ab = float(alpha_bil)
    src, tgt = ei[0].astype(np.int64), ei[1].astype(np.int64)
    n_bins = N_CORES * N_WIN

    # --- balanced assignment of nodes to (core, window) bins ---
    deg = np.bincount(tgt, minlength=N_NODES)
    order = np.argsort(-deg, kind="stable")
    bin_load = np.zeros(n_bins, np.int64)
    bin_fill = np.zeros(n_bins, np.int64)
    node_bin = np.empty(N_NODES, np.int64)
    node_slot = np.empty(N_NODES, np.int64)
    import heapq
    heap = [(0, b) for b in range(n_bins)]
    heapq.heapify(heap)
    for n in order:
        while True:
            load, b = heapq.heappop(heap)
            if bin_fill[b] < W:
                break
        node_bin[n] = b
        node_slot[n] = bin_fill[b]
        bin_fill[b] += 1
        bin_load[b] = load + deg[n]
        if bin_fill[b] < W:
            heapq.heappush(heap, (int(bin_load[b]), b))
    g_w_all = np.ceil(bin_load.reshape(N_CORES, N_WIN) / 128).astype(np.int64)
    g_w = np.maximum(1, g_w_all.max(axis=0))
    tot_g = int(g_w.sum())
    g_offs = np.concatenate([[0], np.cumsum(g_w)]).astype(int)

    node_core = node_bin // N_WIN
    node_win = node_bin % N_WIN
    node_local = node_win * W + node_slot          # in [0, N_CPAD)

    # --- per-core edge slots ---
    e_core = node_core[tgt]
    tot_idx = tot_g * 128
    idx_all = np.zeros((N_CORES, tot_idx), np.int32)
    col_all = np.full((N_CORES, tot_idx), -1.0, np.float32)
    eorder = np.lexsort((tgt, node_win[tgt], e_core))
    src_s = src[eorder]; core_s = e_core[eorder]; win_s = node_win[tgt][eorder]
    tl_s = node_slot[tgt][eorder]                  # slot within window
    counts = np.zeros((N_CORES, N_WIN), np.int64)
    np.add.at(counts, (core_s, win_s), 1)
    run_starts = np.zeros((N_CORES, N_WIN), np.int64)
    np.cumsum(counts.ravel()[:-1], out=run_starts.ravel()[1:])
    for c in range(N_CORES):
        for w in range(N_WIN):
            cnt = int(counts[c, w]); s0 = int(run_starts[c, w])
            base = g_offs[w] * 128
            idx_all[c, base:base + cnt] = src_s[s0:s0 + cnt].astype(np.int32)
            col_all[c, base:base + cnt] = tl_s[s0:s0 + cnt].astype(np.float32)

    # --- t-axis permutation: sort triples by (ck-block, gray(ci-block)) so
    # adjacent classes share the ci-block at one boundary: fewest mixed
    # (t-block, h) weight blocks for both Q and P ---
    kb = (ck // 128).astype(np.int64)
    ib = (ci // 128).astype(np.int64)
    perm = np.lexsort((ib ^ kb, kb))
    cip, cjp, ckp = ci[perm], cj[perm], ck[perm]
    cvp = cv[perm]
    scale = ab * am
    Q1 = np.zeros((D_PAD, TS), np.float32)
    Q1[cip, np.arange(NNZ)] = 1.0
    P1 = np.zeros((TS, D_PAD), np.float32)
    P1[np.arange(NNZ), ckp] = cvp * scale
    qmask = [[bool(np.any(Q1[h * 128:(h + 1) * 128,
                             m * 128:(m + 1) * 128]))
              for m in range(NT)] for h in range(2)]
    pmask = [[bool(np.any(P1[m * 128:(m + 1) * 128,
                             h * 128:(h + 1) * 128]))
              for h in range(2)] for m in range(NT)]
    qp = [Q1[h * 128:(h + 1) * 128, :] for h in range(2)]
    qarr_h = np.concatenate(qp, axis=1).astype(e3)        # [128, 2*TS]
    pp_ = [P1[m * 128:(m + 1) * 128, :] for m in range(NT)]
    cpackb = np.concatenate(pp_, axis=1).astype(bf)       # [128, NT*D_PAD]

    # --- packed per-group [G(256) | H(64)] e3m4 tables ---
    GHW = D_PAD + W
    iota = np.arange(W, dtype=np.float32)

    # --- gathered source rows (e3m4) + per-core tensors ---
    fpad = np.zeros((N_NODES, D_PAD), e3)
    fpad[:, :D] = F.astype(e3)
    inv = np.full((N_CORES, N_CPAD), -1, np.int64)
    inv[node_core, node_local] = np.arange(N_NODES)
    Fbf = F.astype(bf)

    in_maps = []
    for c in range(N_CORES):
        gh = np.empty((tot_g, 128, GHW), e3)
        gh[:, :, :D_PAD] = fpad[idx_all[c]].reshape(tot_g, 128, D_PAD)
        tcols = col_all[c].reshape(tot_g, 128)     # [tot_g, 128]
        gh[:, :, D_PAD:] = (tcols[:, :, None]
                            == iota[None, None, :]).astype(e3)
        ghr_c = np.ascontiguousarray(
            gh.transpose(1, 0, 2).reshape(128, tot_g * GHW))
        sel = inv[c]
        valid = sel >= 0
        fv = F[sel[valid]]                         # [nvalid, D]
        fgy_c = np.zeros((TS, N_CPAD), bf)
        fgy_c[:NNZ, valid] = fv[:, cjp].T.astype(bf)
        # chunk-major: [128, NCH, NT, CW]
        fgy_r = fgy_c.reshape(NT, 128, NCH, CW)
        fgy_cm = np.ascontiguousarray(
            fgy_r.transpose(1, 2, 0, 3).reshape(128, NCH * NT * CW))
        in_maps.append({
            "ghr": ghr_c,
            "qarr": qarr_h,
            "cpb": cpackb,
            "fgy": fgy_cm,
        })
    return (tuple(g_w.tolist()),
            tuple(tuple(r) for r in qmask), tuple(tuple(r) for r in pmask),
            in_maps, inv, F)


def _run(in_maps, inv, fref, nc, trace=False):
    res = run_bass_kernel_spmd(nc, in_maps, core_ids=list(range(N_CORES)),
                               trace=trace)
    out = np.empty((N_NODES, D), np.float32)
    for c in range(N_CORES):
        sel = inv[c]
        valid = sel >= 0
        arr = res.results[c]["out"]              # [128, 2*N_CPAD] bf16 agg^T
        full = np.concatenate([arr[:, :N_CPAD], arr[:, N_CPAD:]], axis=0)
        out[sel[valid]] = (full[:D, valid].T.astype(np.float32)
                           + fref[sel[valid]])
    return out, res


def _get(inputs):
    g_w, qmask, pmask, in_maps, inv, fref = _prep(**inputs)
    key = (g_w, qmask, pmask)
    if key not in _CACHE:
        _CACHE[key] = _build(np.array(g_w), [list(r) for r in qmask],
                             [list(r) for r in pmask])
    return in_maps, inv, fref, _CACHE[key]


def kernel(**inputs):
    in_maps, inv, fref, nc = _get(inputs)
    out, _ = _run(in_maps, inv, fref, nc, trace=False)
    return out


def kernel_traced(**inputs):
    in_maps, inv, fref, nc = _get(inputs)
    return _run(in_maps, inv, fref, nc, trace=True)
